# revision 4
# baseline (speedup 1.0000x reference)
"""CompGCN layer on 8 Trainium2 NeuronCores.

Strategy: sort edges by destination node, shard destination nodes across the 8
cores (6250 nodes each).  Each core gathers ent/-rel embedding rows on-device
via indirect DMA, scatter-adds them per (node-tile, direction) with one-hot
fp16 matmuls accumulating in PSUM, applies the per-direction weights with a
second fp16 matmul, computes BN statistics with ones-vector matmuls, all-reduces
the [1,256] stats across cores, then normalizes + relu and writes its 6250-row
output slice.  rel_out is computed redundantly on every core (tiny).
"""
import sys

sys.path.insert(0, "/root/problem")  # irfix/axon_prof when run from elsewhere
import numpy as np

import concourse.bass as bass
import concourse.mybir as mybir
import concourse.tile as tile
from concourse.bass_utils import run_bass_kernel_spmd
from concourse.masks import make_identity

try:
    from irfix import fix_multiwait
except ImportError:
    # self-contained fallback (harness runs kernel.py alone)
    def fix_multiwait(nc):
        n_fixed = 0
        for func in nc.m.functions:
            for block in func.blocks:
                insts = block.instructions
                i = 0
                while i < len(insts):
                    inst = insts[i]
                    si = inst.sync_info
                    if si is not None and si.on_wait is not None and len(si.on_wait) > 1:
                        waits = list(si.on_wait)
                        for j, w in enumerate(waits[:-1]):
                            nop = mybir.InstNoOp(
                                name=f"{inst.name}_w{j}",
                                engine=inst.engine,
                                sync_info=mybir.SyncInfo(on_wait=[w], on_update=[]),
                            )
                            insts.insert(i, nop)
                            i += 1
                        si.on_wait = [waits[-1]]
                        inst.sync_info = si
                        n_fixed += 1
                    i += 1
                block.instructions = insts
        return n_fixed


N_CORES = 8
P = 128
BN_EPS = 1e-5
F16 = mybir.dt.float16
F32 = mybir.dt.float32
I32 = mybir.dt.int32
I16 = mybir.dt.int16


def _host_prep(ent_emb, edge_index, edge_type, edge_dir, edge_norm):
    N = ent_emb.shape[0]
    E = edge_index.shape[1]
    NPC = N // N_CORES  # nodes per core
    NT = (NPC + P - 1) // P  # node tiles per core

    src = edge_index[0].astype(np.int64)
    dst = edge_index[1].astype(np.int64)
    core = dst // NPC
    local = dst - core * NPC
    nt = local // P
    code = local - nt * P
    d = edge_dir.astype(np.int64)

    ngroups = N_CORES * NT * 3
    key = (core * NT + nt) * 3 + d
    cnt = np.bincount(key, minlength=ngroups)
    Tfix = int(np.ceil(cnt.max() / P))
    NTILES = NT * 3 * Tfix

    order = np.argsort(key, kind="stable")
    sk = key[order]
    starts = np.zeros(ngroups, dtype=np.int64)
    np.cumsum(cnt[:-1], out=starts[1:])
    rank = np.arange(E, dtype=np.int64) - starts[sk]
    t_slot = rank // P
    lane = rank % P
    col = (nt[order] * 3 + d[order]) * Tfix + t_slot

    idx_src = np.zeros((N_CORES, P, NTILES), np.int32)
    idx_rel = np.zeros((N_CORES, P, NTILES), np.int32)
    wmeta = np.zeros((N_CORES, P, NTILES), np.float16)
    cmeta = np.zeros((N_CORES, P, NTILES), np.float16)
    cs = core[order]
    idx_src[cs, lane, col] = src[order]
    idx_rel[cs, lane, col] = edge_type[order]
    wmeta[cs, lane, col] = edge_norm[order].astype(np.float16)
    cmeta[cs, lane, col] = code[order].astype(np.float16)
    return NPC, NT, Tfix, NTILES, idx_src, idx_rel, wmeta, cmeta


def _build(N, R, NPC, NT, Tfix, NTILES):
    D = 128
    RCH = (R + P - 1) // P  # rel row chunks (4 for R=500)
    nc = bass.Bass()
    ent_d = nc.dram_tensor("ent", [N, D], F32, kind="ExternalInput")
    rel_d = nc.dram_tensor("rel", [R, D], F32, kind="ExternalInput")
    isrc_d = nc.dram_tensor("isrc", [P, NTILES], I32, kind="ExternalInput")
    irel_d = nc.dram_tensor("irel", [P, NTILES], I32, kind="ExternalInput")
    w_d = nc.dram_tensor("wmeta", [P, NTILES], F16, kind="ExternalInput")
    c_d = nc.dram_tensor("cmeta", [P, NTILES], F16, kind="ExternalInput")
    W_d = {}
    for wn in ("W_out", "W_in", "W_loop", "W_rel"):
        W_d[wn] = nc.dram_tensor(wn, [D, D], F32, kind="ExternalInput")
    gam_d = nc.dram_tensor("gamma", [1, D], F32, kind="ExternalInput")
    bet_d = nc.dram_tensor("beta", [1, D], F32, kind="ExternalInput")
    out_d = nc.dram_tensor("out_slice", [NPC, D], F32, kind="ExternalOutput")
    rout_d = nc.dram_tensor("rel_out", [R, D], F32, kind="ExternalOutput")
    nrel_d = nc.dram_tensor("nrel16", [R, D], F16, kind="Internal")

    with tile.TileContext(nc) as tc:
        with tc.tile_pool(name="const", bufs=1) as cpool, tc.tile_pool(
            name="setup_ps", bufs=1, space="PSUM"
        ) as spsum, tc.tile_pool(name="dram", bufs=1, space="DRAM") as dpool:
            ident = cpool.tile([P, P], F32)
            make_identity(nc, ident[:])
            iota_i = cpool.tile([P, P], I16)
            nc.gpsimd.iota(iota_i[:], pattern=[[1, P]], base=0, channel_multiplier=0)
            iota_f = cpool.tile([P, P], F16)
            nc.vector.tensor_copy(out=iota_f[:], in_=iota_i[:])

            # metadata -> SBUF
            isrc = cpool.tile([P, NTILES], I32)
            nc.sync.dma_start(out=isrc[:], in_=isrc_d[:])
            irel = cpool.tile([P, NTILES], I32)
            nc.sync.dma_start(out=irel[:], in_=irel_d[:])
            wme = cpool.tile([P, NTILES], F16)
            nc.sync.dma_start(out=wme[:], in_=w_d[:])
            cme = cpool.tile([P, NTILES], F16)
            nc.sync.dma_start(out=cme[:], in_=c_d[:])

            # weights: load, transpose (PE), cast to fp16 -> WT[k=in, m/f=out]
            WT = {}
            for wn in ("W_out", "W_in", "W_loop", "W_rel"):
                wsb = cpool.tile([P, P], F32, tag="wload")
                nc.sync.dma_start(out=wsb[:], in_=W_d[wn][:])
                wps = spsum.tile([P, P], F32, tag="wtp")
                nc.tensor.transpose(out=wps[:], in_=wsb[:], identity=ident[:])
                wt = cpool.tile([P, P], F16, tag=f"WT_{wn}")
                nc.vector.tensor_copy(out=wt[:], in_=wps[:])
                WT[wn] = wt

            # rel table: load chunks, build -rel fp16 table in DRAM and
            # rel^T fp16 in SBUF (for rel_out)
            relT = cpool.tile([P, RCH * P], F16)
            for c in range(RCH):
                rows = min(P, R - c * P)
                rsb = cpool.tile([P, P], F32, tag="rload")
                if rows < P:
                    nc.vector.memset(rsb[:], 0.0)
                nc.sync.dma_start(out=rsb[:rows], in_=rel_d[c * P : c * P + rows, :])
                nrsb = cpool.tile([P, P], F16, tag="nrel")
                nc.vector.tensor_scalar_mul(nrsb[:rows], rsb[:rows], -1.0)
                nc.sync.dma_start(out=nrel_d[c * P : c * P + rows, :], in_=nrsb[:rows])
                rps = spsum.tile([P, P], F32, tag="rtp")
                nc.tensor.transpose(out=rps[:], in_=rsb[:], identity=ident[:])
                nc.vector.tensor_copy(out=relT[:, c * P : (c + 1) * P], in_=rps[:])

            gam = cpool.tile([1, D], F32)
            nc.sync.dma_start(out=gam[:], in_=gam_d[:])
            bet = cpool.tile([1, D], F32)
            nc.sync.dma_start(out=bet[:], in_=bet_d[:])
            ones_col = cpool.tile([P, 1], F32)
            nc.vector.memset(ones_col[:], 1.0)
            ones_row = cpool.tile([1, P], F32)
            nc.vector.memset(ones_row[:], 1.0)
            eps_t = cpool.tile([1, 1], F32)
            nc.vector.memset(eps_t[:], BN_EPS)

            outbuf = cpool.tile([P, NT * P], F32)  # out^... [node,dim] tiles side by side

            with tc.tile_pool(name="work", bufs=8) as wp, tc.tile_pool(
                name="acc_ps", bufs=2, space="PSUM"
            ) as apsum, tc.tile_pool(
                name="op_ps", bufs=1, space="PSUM"
            ) as opsum, tc.tile_pool(name="st_ps", bufs=1, space="PSUM") as stpsum:
                stats_ps = stpsum.tile([1, 2 * D], F32)
                AT = {}
                for nt in range(NT):
                    for d in range(3):
                        ps = apsum.tile([P, P], F32, tag="pA")
                        base = (nt * 3 + d) * Tfix
                        for t in range(Tfix):
                            col = base + t
                            ge32 = wp.tile([P, P], F32, tag="ge32")
                            nc.gpsimd.indirect_dma_start(
                                out=ge32[:],
                                out_offset=None,
                                in_=ent_d[:],
                                in_offset=bass.IndirectOffsetOnAxis(
                                    ap=isrc[:, col : col + 1], axis=0
                                ),
                            )
                            ge = wp.tile([P, P], F16, tag="ge")
                            nc.vector.tensor_copy(out=ge[:], in_=ge32[:])
                            gr = wp.tile([P, P], F16, tag="gr")
                            nc.gpsimd.indirect_dma_start(
                                out=gr[:],
                                out_offset=None,
                                in_=nrel_d[:],
                                in_offset=bass.IndirectOffsetOnAxis(
                                    ap=irel[:, col : col + 1], axis=0
                                ),
                            )
                            oh = wp.tile([P, P], F16, tag="oh")
                            nc.vector.tensor_tensor(
                                out=oh[:],
                                in0=iota_f[:],
                                in1=cme[:, col : col + 1].to_broadcast([P, P]),
                                op=mybir.AluOpType.is_equal,
                            )
                            nc.vector.tensor_tensor(
                                out=oh[:],
                                in0=oh[:],
                                in1=wme[:, col : col + 1].to_broadcast([P, P]),
                                op=mybir.AluOpType.mult,
                            )
                            nc.tensor.matmul(
                                out=ps[:], lhsT=ge[:], rhs=oh[:],
                                start=(t == 0), stop=False,
                            )
                            nc.tensor.matmul(
                                out=ps[:], lhsT=gr[:], rhs=oh[:],
                                start=False, stop=(t == Tfix - 1),
                            )
                        at = wp.tile([P, P], F16, tag=f"AT{d}")
                        nc.vector.tensor_copy(out=at[:], in_=ps[:])
                        AT[d] = at
                    ops = opsum.tile([P, P], F32, tag="ops")
                    for d, wn in enumerate(("W_out", "W_in", "W_loop")):
                        nc.tensor.matmul(
                            out=ops[:], lhsT=AT[d][:], rhs=WT[wn][:],
                            start=(d == 0), stop=(d == 2),
                        )
                    osl = outbuf[:, nt * P : (nt + 1) * P]
                    nc.vector.tensor_copy(out=osl, in_=ops[:])
                    sq = wp.tile([P, P], F32, tag="sq")
                    nc.vector.tensor_tensor(
                        out=sq[:], in0=osl, in1=osl, op=mybir.AluOpType.mult
                    )
                    nc.tensor.matmul(
                        out=stats_ps[:, 0:D], lhsT=ones_col[:], rhs=osl,
                        start=(nt == 0), stop=(nt == NT - 1),
                    )
                    nc.tensor.matmul(
                        out=stats_ps[:, D : 2 * D], lhsT=ones_col[:], rhs=sq[:],
                        start=(nt == 0), stop=(nt == NT - 1),
                    )

                # ---- BN stats all-reduce ----
                stats_sb = cpool.tile([1, 2 * D], F32)
                nc.vector.tensor_copy(out=stats_sb[:], in_=stats_ps[:])
                cc_in = dpool.tile([1, 2 * D], F32)
                cc_out = dpool.tile([1, 2 * D], F32)
                nc.gpsimd.dma_start(cc_in[:], stats_sb[:])
                nc.gpsimd.collective_compute(
                    "AllReduce",
                    mybir.AluOpType.add,
                    replica_groups=[list(range(N_CORES))],
                    ins=[cc_in.opt()],
                    outs=[cc_out.opt()],
                )
                sums = cpool.tile([1, 2 * D], F32)
                nc.gpsimd.dma_start(sums[:], cc_out[:])

                mean = cpool.tile([1, D], F32)
                nc.vector.tensor_scalar_mul(mean[:], sums[:, 0:D], 1.0 / N)
                ex2 = cpool.tile([1, D], F32)
                nc.vector.tensor_scalar_mul(ex2[:], sums[:, D : 2 * D], 1.0 / N)
                var = cpool.tile([1, D], F32)
                nc.vector.tensor_tensor(
                    out=var[:], in0=mean[:], in1=mean[:], op=mybir.AluOpType.mult
                )
                nc.vector.tensor_tensor(
                    out=var[:], in0=ex2[:], in1=var[:], op=mybir.AluOpType.subtract
                )
                stdt = cpool.tile([1, D], F32)
                nc.scalar.activation(
                    stdt[:], var[:], mybir.ActivationFunctionType.Sqrt, bias=eps_t[:]
                )
                rstd = cpool.tile([1, D], F32)
                nc.vector.reciprocal(rstd[:], stdt[:])
                ab = cpool.tile([1, 2 * D], F32)
                nc.vector.tensor_tensor(
                    out=ab[:, 0:D], in0=rstd[:], in1=gam[:], op=mybir.AluOpType.mult
                )  # alpha
                malpha = cpool.tile([1, D], F32)
                nc.vector.tensor_tensor(
                    out=malpha[:], in0=mean[:], in1=ab[:, 0:D], op=mybir.AluOpType.mult
                )
                nc.vector.tensor_tensor(
                    out=ab[:, D : 2 * D], in0=bet[:], in1=malpha[:],
                    op=mybir.AluOpType.subtract,
                )  # beta - mean*alpha
                bc_ps = stpsum.tile([P, 2 * D], F32, tag="bc")
                nc.tensor.matmul(
                    out=bc_ps[:], lhsT=ones_row[:], rhs=ab[:], start=True, stop=True
                )
                abc = cpool.tile([P, 2 * D], F32)
                nc.vector.tensor_copy(out=abc[:], in_=bc_ps[:])

                # ---- normalize + relu + store ----
                for nt in range(NT):
                    rows = min(P, NPC - nt * P)
                    y1 = wp.tile([P, P], F32, tag="y1")
                    nc.vector.tensor_tensor(
                        out=y1[:], in0=outbuf[:, nt * P : (nt + 1) * P],
                        in1=abc[:, 0:D], op=mybir.AluOpType.mult,
                    )
                    nc.vector.tensor_tensor(
                        out=y1[:], in0=y1[:], in1=abc[:, D : 2 * D],
                        op=mybir.AluOpType.add,
                    )
                    nc.vector.tensor_scalar_max(y1[:], y1[:], 0.0)
                    nc.sync.dma_start(
                        out=out_d[nt * P : nt * P + rows, :], in_=y1[:rows, :]
                    )

                # ---- rel_out = relu(rel @ W_rel^T) ----
                for c in range(RCH):
                    rows = min(P, R - c * P)
                    rps2 = opsum.tile([P, P], F32, tag="ro")
                    nc.tensor.matmul(
                        out=rps2[:rows, :],
                        lhsT=relT[:, c * P : c * P + rows],
                        rhs=WT["W_rel"][:],
                        start=True, stop=True,
                    )
                    ro = wp.tile([P, P], F32, tag="rosb")
                    nc.vector.tensor_copy(out=ro[:rows, :], in_=rps2[:rows, :])
                    nc.vector.tensor_scalar_max(ro[:rows, :], ro[:rows, :], 0.0)
                    nc.sync.dma_start(
                        out=rout_d[c * P : c * P + rows, :], in_=ro[:rows, :]
                    )

    fix_multiwait(nc)
    return nc


def kernel(ent_emb, rel_emb, edge_index, edge_type, edge_dir, edge_norm,
           W_out, W_in, W_loop, W_rel, bn_gamma, bn_beta):
    ent_emb = np.asarray(ent_emb, np.float32)
    rel_emb = np.asarray(rel_emb, np.float32)
    edge_index = np.asarray(edge_index, np.int32)
    edge_type = np.asarray(edge_type, np.int32)
    edge_dir = np.asarray(edge_dir, np.int32)
    edge_norm = np.asarray(edge_norm, np.float32)
    N, D = ent_emb.shape
    R = rel_emb.shape[0]

    NPC, NT, Tfix, NTILES, idx_src, idx_rel, wmeta, cmeta = _host_prep(
        ent_emb, edge_index, edge_type, edge_dir, edge_norm
    )
    nc = _build(N, R, NPC, NT, Tfix, NTILES)

    in_maps = []
    for m in range(N_CORES):
        in_maps.append(
            {
                "ent": ent_emb,
                "rel": rel_emb,
                "isrc": idx_src[m],
                "irel": idx_rel[m],
                "wmeta": wmeta[m],
                "cmeta": cmeta[m],
                "W_out": np.asarray(W_out, np.float32),
                "W_in": np.asarray(W_in, np.float32),
                "W_loop": np.asarray(W_loop, np.float32),
                "W_rel": np.asarray(W_rel, np.float32),
                "gamma": np.asarray(bn_gamma, np.float32).reshape(1, D),
                "beta": np.asarray(bn_beta, np.float32).reshape(1, D),
            }
        )
    res = run_bass_kernel_spmd(nc, in_maps, core_ids=list(range(N_CORES)))
    out = np.concatenate([res.results[m]["out_slice"] for m in range(N_CORES)], axis=0)
    rel_out = res.results[0]["rel_out"]
    return out, rel_out


# revision 8
# speedup vs baseline: 1.0014x; 1.0014x over previous
"""CompGCN layer on 8 Trainium2 NeuronCores.

Strategy: sort edges by destination node, shard destination nodes across the 8
cores (6250 nodes each).  Each core gathers ent/-rel embedding rows on-device
via indirect DMA, scatter-adds them per (node-tile, direction) with one-hot
fp16 matmuls accumulating in PSUM, applies the per-direction weights with a
second fp16 matmul, computes BN statistics with ones-vector matmuls, all-reduces
the [1,256] stats across cores, then normalizes + relu and writes its 6250-row
output slice.  rel_out is computed redundantly on every core (tiny).
"""
import sys

sys.path.insert(0, "/root/problem")  # irfix/axon_prof when run from elsewhere
import numpy as np

import concourse.bass as bass
import concourse.mybir as mybir
import concourse.tile as tile
from concourse.bass_utils import run_bass_kernel_spmd
from concourse.masks import make_identity

try:
    from irfix import fix_multiwait
except ImportError:
    # self-contained fallback (harness runs kernel.py alone)
    def fix_multiwait(nc):
        n_fixed = 0
        for func in nc.m.functions:
            for block in func.blocks:
                insts = block.instructions
                i = 0
                while i < len(insts):
                    inst = insts[i]
                    si = inst.sync_info
                    if si is not None and si.on_wait is not None and len(si.on_wait) > 1:
                        waits = list(si.on_wait)
                        for j, w in enumerate(waits[:-1]):
                            nop = mybir.InstNoOp(
                                name=f"{inst.name}_w{j}",
                                engine=inst.engine,
                                sync_info=mybir.SyncInfo(on_wait=[w], on_update=[]),
                            )
                            insts.insert(i, nop)
                            i += 1
                        si.on_wait = [waits[-1]]
                        inst.sync_info = si
                        n_fixed += 1
                    i += 1
                block.instructions = insts
        return n_fixed


N_CORES = 8
P = 128
BN_EPS = 1e-5
F16 = mybir.dt.float16
F32 = mybir.dt.float32
I32 = mybir.dt.int32
I16 = mybir.dt.int16


def _host_prep(ent_emb, edge_index, edge_type, edge_dir, edge_norm):
    N = ent_emb.shape[0]
    E = edge_index.shape[1]
    NPC = N // N_CORES  # nodes per core
    NT = (NPC + P - 1) // P  # node tiles per core

    src = edge_index[0].astype(np.int64)
    dst = edge_index[1].astype(np.int64)
    core = dst // NPC
    local = dst - core * NPC
    nt = local // P
    code = local - nt * P
    d = edge_dir.astype(np.int64)

    ngroups = N_CORES * NT * 3
    key = (core * NT + nt) * 3 + d
    cnt = np.bincount(key, minlength=ngroups)
    Tfix = int(np.ceil(cnt.max() / P))
    NTILES = NT * 3 * Tfix

    order = np.argsort(key, kind="stable")
    sk = key[order]
    starts = np.zeros(ngroups, dtype=np.int64)
    np.cumsum(cnt[:-1], out=starts[1:])
    rank = np.arange(E, dtype=np.int64) - starts[sk]
    t_slot = rank // P
    lane = rank % P
    col = (nt[order] * 3 + d[order]) * Tfix + t_slot

    idx_src = np.zeros((N_CORES, P, NTILES), np.int32)
    idx_rel = np.zeros((N_CORES, P, NTILES), np.int32)
    wmeta = np.zeros((N_CORES, P, NTILES), np.float32)
    cmeta = np.zeros((N_CORES, P, NTILES), np.float32)
    cs = core[order]
    idx_src[cs, lane, col] = src[order]
    idx_rel[cs, lane, col] = edge_type[order]
    wmeta[cs, lane, col] = edge_norm[order]
    cmeta[cs, lane, col] = code[order].astype(np.float32)

    # rel indices in dma_gather layout: per group g, slot i=t*128+lane;
    # idx list wrapped over 16 partitions (i%16) and replicated 8x.
    NT3 = NT * 3
    C = Tfix * 128 // 16  # int16 cols per group
    L_all = (
        idx_rel.reshape(N_CORES, P, NT3, Tfix)
        .transpose(0, 2, 3, 1)
        .reshape(N_CORES, NT3, Tfix * P)
        .astype(np.int16)
    )
    A = L_all.reshape(N_CORES, NT3, C, 16)
    B = A.transpose(0, 3, 1, 2).reshape(N_CORES, 16, NT3 * C)
    relg = np.tile(B, (1, 8, 1))  # [NC, 128, NT3*C]
    return NPC, NT, Tfix, NTILES, idx_src, idx_rel, wmeta, cmeta, relg


def _build(N, R, NPC, NT, Tfix, NTILES):
    D = 128
    RCH = (R + P - 1) // P  # rel row chunks (4 for R=500)
    nc = bass.Bass()
    ent_d = nc.dram_tensor("ent", [N, D], F32, kind="ExternalInput")
    rel_d = nc.dram_tensor("rel", [R, D], F32, kind="ExternalInput")
    isrc_d = nc.dram_tensor("isrc", [P, NTILES], I32, kind="ExternalInput")
    irel_d = nc.dram_tensor("irel", [P, NTILES], I32, kind="ExternalInput")
    w_d = nc.dram_tensor("wmeta", [P, NTILES], F32, kind="ExternalInput")
    c_d = nc.dram_tensor("cmeta", [P, NTILES], F32, kind="ExternalInput")
    RC = Tfix * 128 // 16
    relg_d = nc.dram_tensor("relg", [P, NT * 3 * RC], I16, kind="ExternalInput")
    W_d = {}
    for wn in ("W_out", "W_in", "W_loop", "W_rel"):
        W_d[wn] = nc.dram_tensor(wn, [D, D], F32, kind="ExternalInput")
    gam_d = nc.dram_tensor("gamma", [1, D], F32, kind="ExternalInput")
    bet_d = nc.dram_tensor("beta", [1, D], F32, kind="ExternalInput")
    out_d = nc.dram_tensor("out_slice", [NPC, D], F32, kind="ExternalOutput")
    rout_d = nc.dram_tensor("rel_out", [R, D], F32, kind="ExternalOutput")
    nrel_d = nc.dram_tensor("nrel16", [R, D], F16, kind="Internal")

    with tile.TileContext(nc) as tc:
        with tc.tile_pool(name="const", bufs=1) as cpool, tc.tile_pool(
            name="setup_ps", bufs=1, space="PSUM"
        ) as spsum, tc.tile_pool(name="dram", bufs=1, space="DRAM") as dpool:
            ident = cpool.tile([P, P], F32)
            make_identity(nc, ident[:])
            iota_i = cpool.tile([P, P], I16)
            nc.gpsimd.iota(iota_i[:], pattern=[[1, P]], base=0, channel_multiplier=0)
            iota_f = cpool.tile([P, P], F16)
            nc.vector.tensor_copy(out=iota_f[:], in_=iota_i[:])

            # metadata -> SBUF
            isrc = cpool.tile([P, NTILES], I32)
            nc.sync.dma_start(out=isrc[:], in_=isrc_d[:])
            irel = cpool.tile([P, NTILES], I32)
            nc.sync.dma_start(out=irel[:], in_=irel_d[:])
            wme = cpool.tile([P, NTILES], F32)
            nc.sync.dma_start(out=wme[:], in_=w_d[:])
            cme = cpool.tile([P, NTILES], F32)
            nc.sync.dma_start(out=cme[:], in_=c_d[:])


            # weights: load, transpose (PE), cast to fp16 -> WT[k=in, m/f=out]
            WT = {}
            for wn in ("W_out", "W_in", "W_loop", "W_rel"):
                wsb = cpool.tile([P, P], F32, tag="wload")
                nc.sync.dma_start(out=wsb[:], in_=W_d[wn][:])
                wps = spsum.tile([P, P], F32, tag="wtp")
                nc.tensor.transpose(out=wps[:], in_=wsb[:], identity=ident[:])
                wt = cpool.tile([P, P], F16, tag=f"WT_{wn}")
                nc.vector.tensor_copy(out=wt[:], in_=wps[:])
                WT[wn] = wt

            # rel table: load chunks, build -rel fp16 table in DRAM and
            # rel^T fp16 in SBUF (for rel_out)
            relT = cpool.tile([P, RCH * P], F16)
            for c in range(RCH):
                rows = min(P, R - c * P)
                rsb = cpool.tile([P, P], F32, tag="rload")
                if rows < P:
                    nc.vector.memset(rsb[:], 0.0)
                nc.sync.dma_start(out=rsb[:rows], in_=rel_d[c * P : c * P + rows, :])
                nrsb = cpool.tile([P, P], F16, tag="nrel")
                nc.vector.tensor_scalar_mul(nrsb[:rows], rsb[:rows], -1.0)
                nc.sync.dma_start(out=nrel_d[c * P : c * P + rows, :], in_=nrsb[:rows])
                rps = spsum.tile([P, P], F32, tag="rtp")
                nc.tensor.transpose(out=rps[:], in_=rsb[:], identity=ident[:])
                nc.vector.tensor_copy(out=relT[:, c * P : (c + 1) * P], in_=rps[:])

            gam = cpool.tile([1, D], F32)
            nc.sync.dma_start(out=gam[:], in_=gam_d[:])
            bet = cpool.tile([1, D], F32)
            nc.sync.dma_start(out=bet[:], in_=bet_d[:])
            ones_col = cpool.tile([P, 1], F32)
            nc.vector.memset(ones_col[:], 1.0)
            ones_row = cpool.tile([1, P], F32)
            nc.vector.memset(ones_row[:], 1.0)
            eps_t = cpool.tile([1, 1], F32)
            nc.vector.memset(eps_t[:], BN_EPS)

            outbuf = cpool.tile([P, NT * P], F32)
  # out^... [node,dim] tiles side by side

            with tc.tile_pool(name="work", bufs=8) as wp, tc.tile_pool(
                name="acc_ps", bufs=2, space="PSUM"
            ) as apsum, tc.tile_pool(
                name="op_ps", bufs=1, space="PSUM"
            ) as opsum, tc.tile_pool(name="st_ps", bufs=1, space="PSUM") as stpsum:
                stats_ps = stpsum.tile([1, 2 * D], F32)
                AT = {}
                for nt in range(NT):
                    for d in range(3):
                        ps = apsum.tile([P, P], F32, tag="pA")
                        base = (nt * 3 + d) * Tfix
                        for t in range(Tfix):
                            col = base + t
                            ge32 = wp.tile([P, P], F32, tag="ge32")
                            nc.gpsimd.indirect_dma_start(
                                out=ge32[:],
                                out_offset=None,
                                in_=ent_d[:],
                                in_offset=bass.IndirectOffsetOnAxis(
                                    ap=isrc[:, col : col + 1], axis=0
                                ),
                            )
                            ge = wp.tile([P, P], F16, tag="ge")
                            nc.vector.tensor_copy(out=ge[:], in_=ge32[:])
                            oh = wp.tile([P, P], F16, tag="oh")
                            nc.vector.tensor_scalar(
                                out=oh[:],
                                in0=iota_f[:],
                                scalar1=cme[:, col : col + 1],
                                scalar2=wme[:, col : col + 1],
                                op0=mybir.AluOpType.is_equal,
                                op1=mybir.AluOpType.mult,
                            )
                            nc.tensor.matmul(
                                out=ps[:], lhsT=ge[:], rhs=oh[:],
                                start=(t == 0), stop=False,
                            )
                            gr = wp.tile([P, P], F16, tag="gr")
                            nc.gpsimd.indirect_dma_start(
                                out=gr[:],
                                out_offset=None,
                                in_=nrel_d[:],
                                in_offset=bass.IndirectOffsetOnAxis(
                                    ap=irel[:, col : col + 1], axis=0
                                ),
                            )
                            nc.tensor.matmul(
                                out=ps[:], lhsT=gr[:], rhs=oh[:],
                                start=False, stop=(t == Tfix - 1),
                            )
                        at = wp.tile([P, P], F16, tag=f"AT{d}")
                        nc.vector.tensor_copy(out=at[:], in_=ps[:])
                        AT[d] = at
                    ops = opsum.tile([P, P], F32, tag="ops")
                    for d, wn in enumerate(("W_out", "W_in", "W_loop")):
                        nc.tensor.matmul(
                            out=ops[:], lhsT=AT[d][:], rhs=WT[wn][:],
                            start=(d == 0), stop=(d == 2),
                        )
                    osl = outbuf[:, nt * P : (nt + 1) * P]
                    nc.vector.tensor_copy(out=osl, in_=ops[:])
                    sq = wp.tile([P, P], F32, tag="sq")
                    nc.vector.tensor_tensor(
                        out=sq[:], in0=osl, in1=osl, op=mybir.AluOpType.mult
                    )
                    nc.tensor.matmul(
                        out=stats_ps[:, 0:D], lhsT=ones_col[:], rhs=osl,
                        start=(nt == 0), stop=(nt == NT - 1),
                    )
                    nc.tensor.matmul(
                        out=stats_ps[:, D : 2 * D], lhsT=ones_col[:], rhs=sq[:],
                        start=(nt == 0), stop=(nt == NT - 1),
                    )

                # ---- BN stats all-reduce ----
                stats_sb = cpool.tile([1, 2 * D], F32)
                nc.vector.tensor_copy(out=stats_sb[:], in_=stats_ps[:])
                cc_in = dpool.tile([1, 2 * D], F32)
                cc_out = dpool.tile([1, 2 * D], F32)
                nc.gpsimd.dma_start(cc_in[:], stats_sb[:])
                nc.gpsimd.collective_compute(
                    "AllReduce",
                    mybir.AluOpType.add,
                    replica_groups=[list(range(N_CORES))],
                    ins=[cc_in.opt()],
                    outs=[cc_out.opt()],
                )
                sums = cpool.tile([1, 2 * D], F32)
                nc.gpsimd.dma_start(sums[:], cc_out[:])

                mean = cpool.tile([1, D], F32)
                nc.vector.tensor_scalar_mul(mean[:], sums[:, 0:D], 1.0 / N)
                ex2 = cpool.tile([1, D], F32)
                nc.vector.tensor_scalar_mul(ex2[:], sums[:, D : 2 * D], 1.0 / N)
                var = cpool.tile([1, D], F32)
                nc.vector.tensor_tensor(
                    out=var[:], in0=mean[:], in1=mean[:], op=mybir.AluOpType.mult
                )
                nc.vector.tensor_tensor(
                    out=var[:], in0=ex2[:], in1=var[:], op=mybir.AluOpType.subtract
                )
                stdt = cpool.tile([1, D], F32)
                nc.scalar.activation(
                    stdt[:], var[:], mybir.ActivationFunctionType.Sqrt, bias=eps_t[:]
                )
                rstd = cpool.tile([1, D], F32)
                nc.vector.reciprocal(rstd[:], stdt[:])
                ab = cpool.tile([1, 2 * D], F32)
                nc.vector.tensor_tensor(
                    out=ab[:, 0:D], in0=rstd[:], in1=gam[:], op=mybir.AluOpType.mult
                )  # alpha
                malpha = cpool.tile([1, D], F32)
                nc.vector.tensor_tensor(
                    out=malpha[:], in0=mean[:], in1=ab[:, 0:D], op=mybir.AluOpType.mult
                )
                nc.vector.tensor_tensor(
                    out=ab[:, D : 2 * D], in0=bet[:], in1=malpha[:],
                    op=mybir.AluOpType.subtract,
                )  # beta - mean*alpha
                bc_ps = stpsum.tile([P, 2 * D], F32, tag="bc")
                nc.tensor.matmul(
                    out=bc_ps[:], lhsT=ones_row[:], rhs=ab[:], start=True, stop=True
                )
                abc = cpool.tile([P, 2 * D], F32)
                nc.vector.tensor_copy(out=abc[:], in_=bc_ps[:])

                # ---- normalize + relu + store ----
                for nt in range(NT):
                    rows = min(P, NPC - nt * P)
                    y1 = wp.tile([P, P], F32, tag="y1")
                    nc.vector.tensor_tensor(
                        out=y1[:], in0=outbuf[:, nt * P : (nt + 1) * P],
                        in1=abc[:, 0:D], op=mybir.AluOpType.mult,
                    )
                    nc.vector.tensor_tensor(
                        out=y1[:], in0=y1[:], in1=abc[:, D : 2 * D],
                        op=mybir.AluOpType.add,
                    )
                    nc.vector.tensor_scalar_max(y1[:], y1[:], 0.0)
                    nc.sync.dma_start(
                        out=out_d[nt * P : nt * P + rows, :], in_=y1[:rows, :]
                    )

                # ---- rel_out = relu(rel @ W_rel^T) ----
                for c in range(RCH):
                    rows = min(P, R - c * P)
                    rps2 = opsum.tile([P, P], F32, tag="ro")
                    nc.tensor.matmul(
                        out=rps2[:rows, :],
                        lhsT=relT[:, c * P : c * P + rows],
                        rhs=WT["W_rel"][:],
                        start=True, stop=True,
                    )
                    ro = wp.tile([P, P], F32, tag="rosb")
                    nc.vector.tensor_copy(out=ro[:rows, :], in_=rps2[:rows, :])
                    nc.vector.tensor_scalar_max(ro[:rows, :], ro[:rows, :], 0.0)
                    nc.sync.dma_start(
                        out=rout_d[c * P : c * P + rows, :], in_=ro[:rows, :]
                    )

    fix_multiwait(nc)
    return nc


def kernel(ent_emb, rel_emb, edge_index, edge_type, edge_dir, edge_norm,
           W_out, W_in, W_loop, W_rel, bn_gamma, bn_beta):
    ent_emb = np.asarray(ent_emb, np.float32)
    rel_emb = np.asarray(rel_emb, np.float32)
    edge_index = np.asarray(edge_index, np.int32)
    edge_type = np.asarray(edge_type, np.int32)
    edge_dir = np.asarray(edge_dir, np.int32)
    edge_norm = np.asarray(edge_norm, np.float32)
    N, D = ent_emb.shape
    R = rel_emb.shape[0]

    NPC, NT, Tfix, NTILES, idx_src, idx_rel, wmeta, cmeta, relg = _host_prep(
        ent_emb, edge_index, edge_type, edge_dir, edge_norm
    )
    nc = _build(N, R, NPC, NT, Tfix, NTILES)

    in_maps = []
    for m in range(N_CORES):
        in_maps.append(
            {
                "ent": ent_emb,
                "rel": rel_emb,
                "isrc": idx_src[m],
                "irel": idx_rel[m],
                "wmeta": wmeta[m],
                "cmeta": cmeta[m],
                "relg": relg[m],
                "W_out": np.asarray(W_out, np.float32),
                "W_in": np.asarray(W_in, np.float32),
                "W_loop": np.asarray(W_loop, np.float32),
                "W_rel": np.asarray(W_rel, np.float32),
                "gamma": np.asarray(bn_gamma, np.float32).reshape(1, D),
                "beta": np.asarray(bn_beta, np.float32).reshape(1, D),
            }
        )
    res = run_bass_kernel_spmd(nc, in_maps, core_ids=list(range(N_CORES)))
    out = np.concatenate([res.results[m]["out_slice"] for m in range(N_CORES)], axis=0)
    rel_out = res.results[0]["rel_out"]
    return out, rel_out
